# revision 1
# baseline (speedup 1.0000x reference)
"""EpisodicMemory retrieval kernel for Trainium2 (8 NeuronCores, data-parallel).

Reference computation (per row b of query):
    q = query @ Wq.T;  sim = l2norm(q) @ l2norm(keys).T
    top4 vals/idx;  w = softmax(5*vals);  retrieved = sum w_k * V[idx_k]
    projected = retrieved @ Wv.T
    gate = sigmoid([query, projected] @ Wg.T + bg);  out = gate * projected

Device mapping (per core, 2048 rows = 16 tiles of 128):
  - Selection path must preserve the fp32 ranking: sim = query @ Wc with
    Wc = Wq.T @ keys_norm.T folded on host in fp64. The matmul runs as a
    hybrid: one fp32r hi*hi pass (1 cyc/col) plus two fp8 DoubleRow
    residual passes (0.5 cyc/col, 256-deep) capturing lo*hi and hi*lo:
        sim ~= qh.wh + e4m3(ql*2^8).e5m2(wh*2^-8) + e5m2(qh*2^-8).e5m2(wl*2^8)
    The fp8 operand pre-scaling keeps every product at natural scale so
    all passes accumulate into one PSUM bank. Residual noise ~5e-7 on
    cosine sims, a handful of 4-vs-5 boundary swaps over the full batch.
  - top-8 + indices via the DVE Max8/MaxIndex instructions.
  - Vp = V @ Wv.T and Vg = Vp @ Wg2.T + bg are folded on host into one
    fp16 table [E, 2D], gathered per row by the GPSIMD dma_gather as
    int32 words (halves the per-element gather cost on the Pool queue).
  - Weighted sums run on the PE as 4 accumulated fp16 matmuls against
    diag(w_k), producing `projected` and the gate's Z2 term directly.
  - q itself is only needed for ||q|| (softmax temperature): one fp8
    DoubleRow matmul + ACT square with accum_out.
  - Output path (gate, projected, out) is fp16: DVE runs the final
    multiply in 2x mode and the store DMA halves; host casts to fp32.
"""

import sys

sys.path.insert(0, "/opt/trn_rl_repo")

import numpy as np
import ml_dtypes

from concourse import bass, bacc, mybir
from concourse.tile import TileContext
from concourse.bass_utils import run_bass_kernel_spmd

P = 128
D = 512
E = 1024
TOPK = 4
N_CORES = 8
B_FULL = 16384

DT = mybir.dt
F32 = DT.float32
F32R = DT.float32r
FP16 = DT.float16
F8E4 = DT.float8e4
F8E5 = DT.float8e5
I16 = DT.int16
U16 = DT.uint16
I64 = DT.int64

E4 = ml_dtypes.float8_e4m3
E5 = ml_dtypes.float8_e5m2

AF = mybir.ActivationFunctionType
ALU = mybir.AluOpType
DR = mybir.MatmulPerfMode.DoubleRow

# fp8 residual-pass operand pre-scaling (exact powers of two, product = 1)
FS = 256.0


def round_fp32r(x: np.ndarray) -> np.ndarray:
    """Round fp32 to the fp32r grid (1 sign + 8 exp + 11 mantissa bits, RNE)."""
    u = np.ascontiguousarray(x, dtype=np.float32).view(np.uint32).astype(np.uint64)
    r = u + (0x7FF + ((u >> 12) & 1))
    r &= ~np.uint64(0xFFF)
    r = np.minimum(r, 0xFFFFFFFF).astype(np.uint32)
    return r.view(np.float32)


def build_program(nt: int):
    """Build the per-core bass program processing nt row-tiles of 128."""
    bc = nt * P  # rows per core
    tpg = 4 if nt % 4 == 0 else nt  # max tiles per group
    # Full groups of tpg, but the last group split in half so its
    # softmax/phase-B overlaps the final sims instead of serializing
    # after them.
    groups = [(s, tpg) for s in range(0, nt - tpg, tpg)]
    if tpg >= 2 and tpg % 2 == 0:
        groups += [(nt - tpg, tpg // 2), (nt - tpg // 2, tpg // 2)]
    else:
        groups += [(nt - tpg, tpg)]

    nc = bacc.Bacc()

    qh_d = nc.declare_dram_parameter("qT_hi", [D, bc], F32R, isOutput=False)
    qls_d = nc.declare_dram_parameter("qT_ls", [D, bc], F8E4, isOutput=False)
    qhs_d = nc.declare_dram_parameter("qT_hs", [D, bc], F8E5, isOutput=False)
    q8_d = nc.declare_dram_parameter("qT_8", [D, bc], F8E4, isOutput=False)
    wch_d = nc.declare_dram_parameter("Wc_hi", [D, E], F32R, isOutput=False)
    whs_d = nc.declare_dram_parameter("Wc_hs", [D, E], F8E5, isOutput=False)
    wls_d = nc.declare_dram_parameter("Wc_ls", [D, E], F8E5, isOutput=False)
    qr_d = nc.declare_dram_parameter("qT_r", [D, bc], F8E5, isOutput=False)
    q4_d = nc.declare_dram_parameter("qT_4", [D, bc], F8E5, isOutput=False)
    wq8_d = nc.declare_dram_parameter("WqT8", [D, D], F8E4, isOutput=False)
    wg8_d = nc.declare_dram_parameter("Wg8", [D, D], F8E5, isOutput=False)
    wg84_d = nc.declare_dram_parameter("Wg84", [D, D], F8E5, isOutput=False)
    wgr_d = nc.declare_dram_parameter("Wgr", [D, D], F8E5, isOutput=False)
    rep_d = nc.declare_dram_parameter("rep16", [16, P], F32R, isOutput=False)
    ident_d = nc.declare_dram_parameter("ident", [P, P], FP16, isOutput=False)
    vpg_d = nc.declare_dram_parameter("Vpg32", [E, D], DT.int32, isOutput=False)

    out_d = nc.declare_dram_parameter("out", [bc, D], FP16, isOutput=True)

    KC = D // P  # 4 contraction chunks of 128 for fp32r passes

    with TileContext(nc) as tc:
        with (
            tc.tile_pool(name="const", bufs=1) as cpool,
            tc.tile_pool(name="grp", bufs=2) as gpool,
            tc.tile_pool(name="work", bufs=2) as wpool,
            tc.tile_pool(name="dram", bufs=max(nt, 2), space="DRAM") as dpool,
            tc.tile_pool(name="ps_q", bufs=1, space="PSUM") as pp_q,
            tc.tile_pool(name="ps_s", bufs=3, space="PSUM") as pp_s,
            tc.tile_pool(name="ps_o", bufs=1, space="PSUM") as pp_o,
            tc.tile_pool(name="ps_z", bufs=2, space="PSUM") as pp_z,
        ):
            # ---- constants into SBUF ----
            # Spread across dispatch queues; load what tile 0 needs first
            # (wch half 0 on scalar, whs/wls half 0 on vector).
            wch_sb = cpool.tile([P, KC, E], F32R, tag="wch")
            whs_sb = cpool.tile([P, 2, 2, E], F8E5, tag="whs")
            wls_sb = cpool.tile([P, 2, 2, E], F8E5, tag="wls")
            wq8_sb = cpool.tile([P, 2, 2, D], F8E4, tag="wq8")
            wg8_sb = cpool.tile([P, 2, 2, D], F8E5, tag="wg8")
            wg84_sb = cpool.tile([P, 2, 2, D], F8E5, tag="wg84")
            wgr_sb = cpool.tile([P, 2, 2, D], F8E5, tag="wgr")
            # wch in per-(eh, c) chunks, split across the scalar and gpsimd
            # queues so tile 0's matmul operands land in consumption order
            def wch_chunk(eng, c, eh):
                es = slice(eh * D, (eh + 1) * D)
                eng.dma_start(
                    out=wch_sb[:, c, es], in_=wch_d.ap()[c * P : (c + 1) * P, es]
                )

            def w8_half(sb, dram, eh):
                es = slice(eh * D, (eh + 1) * D)
                nc.gpsimd.dma_start(
                    out=sb[:, :, :, es],
                    in_=dram.ap()[:, es].rearrange("(c t p) m -> p c t m", c=2, t=2),
                )

            wch_chunk(nc.scalar, 0, 0)
            wch_chunk(nc.scalar, 1, 0)
            w8_half(whs_sb, whs_d, 0)
            w8_half(wls_sb, wls_d, 0)
            wch_chunk(nc.gpsimd, 2, 0)
            wch_chunk(nc.gpsimd, 3, 0)
            wch_chunk(nc.scalar, 0, 1)
            wch_chunk(nc.scalar, 1, 1)
            wch_chunk(nc.scalar, 2, 1)
            wch_chunk(nc.scalar, 3, 1)
            w8_half(whs_sb, whs_d, 1)
            w8_half(wls_sb, wls_d, 1)
            nc.gpsimd.dma_start(
                out=wq8_sb, in_=wq8_d.ap().rearrange("(c t p) m -> p c t m", c=2, t=2)
            )
            for sb, dram in ((wg8_sb, wg8_d), (wg84_sb, wg84_d), (wgr_sb, wgr_d)):
                nc.gpsimd.dma_start(
                    out=sb, in_=dram.ap().rearrange("(c t p) m -> p c t m", c=2, t=2)
                )
            ident_sb = cpool.tile([P, P], FP16, tag="ident")
            rep_sb = cpool.tile([16, P], F32R, tag="rep16")

            for gi, (s0, tn) in enumerate(groups):
                gn = tn * P
                gs = slice(s0 * P, s0 * P + gn)
                # ---- group inputs ----
                # Group 0: qh tile-0 piece first, then the fp8 residual
                # operands, then the rest — so tile 0's full sim unblocks
                # as early as possible.
                qh_g = gpool.tile([P, KC, gn], F32R, tag="qh")

                def load_q(sb, dram, ts, te):
                    nc.sync.dma_start(
                        out=sb[:, :, :, ts:te] if sb.shape[1] == 2 else sb[:, :, ts:te],
                        in_=dram.ap()[:, s0 * P + ts : s0 * P + te].rearrange(
                            "(c t p) m -> p c t m" if sb.shape[1] == 2 else "(c p) m -> p c m",
                            **({"c": 2, "t": 2} if sb.shape[1] == 2 else {"p": P}),
                        ),
                    )

                qls_g = gpool.tile([P, 2, 2, gn], F8E4, tag="qls")
                qhs_g = gpool.tile([P, 2, 2, gn], F8E5, tag="qhs")
                q8_g = gpool.tile([P, 2, 2, gn], F8E4, tag="q8")
                qr_g = gpool.tile([P, 2, 2, gn], F8E5, tag="qr")
                q4_g = gpool.tile([P, 2, 2, gn], F8E5, tag="q4")
                if gi == 0:
                    # per-tile trios so each tile's sim operands land just
                    # ahead of the PE consuming them
                    for t in range(tn):
                        load_q(qh_g, qh_d, t * P, (t + 1) * P)
                        load_q(qls_g, qls_d, t * P, (t + 1) * P)
                        load_q(qhs_g, qhs_d, t * P, (t + 1) * P)
                else:
                    load_q(qh_g, qh_d, 0, gn)
                    load_q(qls_g, qls_d, 0, gn)
                    load_q(qhs_g, qhs_d, 0, gn)
                load_q(q8_g, q8_d, 0, gn)
                load_q(qr_g, qr_d, 0, gn)
                load_q(q4_g, q4_d, 0, gn)
                if gi == 0:
                    nc.sync.dma_start(out=ident_sb, in_=ident_d.ap())
                    nc.sync.dma_start(out=rep_sb, in_=rep_d.ap())

                normsq_g = gpool.tile([P, tn], F32, tag="normsq")
                top8_g = gpool.tile([P, tn * 8], F32, tag="top8")
                idx8_g = gpool.tile([P, tn * 8], U16, tag="idx8")
                g_tiles = []

                for t in range(tn):
                    ti = s0 + t
                    bs = slice(t * P, (t + 1) * P)

                    # ---- ||q||^2 via fp8 DoubleRow q = query @ Wq.T ----
                    ps_q = pp_q.tile([P, D], F32, tag="q")
                    for cc in range(2):
                        nc.tensor.matmul(
                            ps_q,
                            q8_g[:, cc, :, bs],
                            wq8_sb[:, cc, :, :],
                            start=(cc == 0),
                            stop=(cc == 1),
                            perf_mode=DR,
                        )
                    qsq = wpool.tile([P, D], FP16, tag="qsq")
                    nc.scalar.activation(
                        qsq, ps_q, AF.Square, accum_out=normsq_g[:, t : t + 1]
                    )

                    # ---- sim = query @ Wc (fp32r hi*hi + fp8 DR residuals) ----
                    sim_sb = wpool.tile([P, E], F32, tag="sim")
                    for eh in range(2):
                        ps_s = pp_s.tile([P, D], F32, tag="s")
                        es = slice(eh * D, (eh + 1) * D)
                        for c in range(KC):
                            nc.tensor.matmul(
                                ps_s,
                                qh_g[:, c, bs],
                                wch_sb[:, c, es],
                                start=(c == 0),
                                stop=False,
                            )
                        for cc in range(2):
                            nc.tensor.matmul(
                                ps_s,
                                qls_g[:, cc, :, bs],
                                whs_sb[:, cc, :, es],
                                start=False,
                                stop=False,
                                perf_mode=DR,
                            )
                        for cc in range(2):
                            nc.tensor.matmul(
                                ps_s,
                                qhs_g[:, cc, :, bs],
                                wls_sb[:, cc, :, es],
                                start=False,
                                stop=(cc == 1),
                                perf_mode=DR,
                            )
                        if eh == 0:
                            nc.scalar.copy(sim_sb[:, es], ps_s)
                        else:
                            nc.vector.tensor_copy(sim_sb[:, es], ps_s)

                    # ---- top-8 values + indices ----
                    t8 = slice(t * 8, (t + 1) * 8)
                    nc.vector.max(out=top8_g[:, t8], in_=sim_sb)
                    nc.vector.max_index(
                        out=idx8_g[:, t8], in_max=top8_g[:, t8], in_values=sim_sb
                    )

                    # ---- index shuffle [128b, 4k] -> [16, 32] via DRAM bounce ----
                    # (see baseline notes: the b -> b%16 partition permutation
                    # needs a DRAM hop; indices travel as exact fp32r ints so
                    # the 8x replication for the GPSIMD cores runs on the PE)
                    idxf = wpool.tile([P, 4], F32R, tag="idxf")
                    nc.vector.tensor_copy(idxf, idx8_g[:, t * 8 : t * 8 + 4])
                    scratch = dpool.tile([1, 512], F32R, tag="scr")
                    nc.gpsimd.dma_start(
                        out=scratch.rearrange("o (c k h) -> o h c k", c=16, k=4, h=8),
                        in_=idxf,
                    )
                    gidxf16 = wpool.tile([16, 32], F32R, tag="gidxf16")
                    nc.sync.dma_start(out=gidxf16, in_=scratch)
                    ps_g = pp_q.tile([P, 32], F32, tag="gidx")
                    nc.tensor.matmul(ps_g, rep_sb, gidxf16, start=True, stop=True)
                    gidx = wpool.tile([P, 32], I16, tag="gidx")
                    nc.vector.tensor_copy(gidx, ps_g)

                    # ---- gather merged [Vp | Vg] rows as int32 words ----
                    g32 = wpool.tile([P, TOPK, D], DT.int32, tag="G32")
                    nc.gpsimd.dma_gather(
                        out_ap=g32,
                        in_ap=vpg_d.ap(),
                        idxs_ap=gidx,
                        num_idxs=TOPK * P,
                        num_idxs_reg=TOPK * P,
                        elem_size=D,
                    )
                    g_tiles.append(g32)

                # ---- softmax over top-4 (batched across the group) ----
                nrm = gpool.tile([P, tn], F32, tag="nrm")
                nc.scalar.sqrt(nrm, normsq_g)
                rrec = gpool.tile([P, tn], F32, tag="rrec")
                nc.vector.reciprocal(rrec, nrm)
                s5 = gpool.tile([P, tn], F32, tag="s5")
                nc.vector.tensor_scalar_mul(s5, rrec, 5.0)

                t8v = top8_g.rearrange("p (t k) -> p t k", k=8)
                top4_v = t8v[:, :, 0:4]
                m_v = t8v[:, :, 0:1].to_broadcast([P, tn, 4])
                s5_v = s5.rearrange("p (t o) -> p t o", o=1).to_broadcast([P, tn, 4])

                args = gpool.tile([P, tn * 4], F32, tag="args")
                args_v = args.rearrange("p (t k) -> p t k", k=4)
                nc.vector.tensor_tensor(args_v, top4_v, m_v, op=ALU.subtract)
                nc.vector.tensor_tensor(args_v, args_v, s5_v, op=ALU.mult)
                ex = gpool.tile([P, tn * 4], F32, tag="ex")
                nc.scalar.activation(ex, args, AF.Exp)
                ex_v = ex.rearrange("p (t k) -> p t k", k=4)
                den = gpool.tile([P, tn], F32, tag="den")
                nc.vector.tensor_reduce(den, ex_v, axis=mybir.AxisListType.X, op=ALU.add)
                rden = gpool.tile([P, tn], F32, tag="rden")
                nc.vector.reciprocal(rden, den)
                rden_v = rden.rearrange("p (t o) -> p t o", o=1).to_broadcast(
                    [P, tn, 4]
                )
                w_g = gpool.tile([P, tn * 4], F32, tag="w")
                w_v = w_g.rearrange("p (t k) -> p t k", k=4)
                nc.vector.tensor_tensor(w_v, ex_v, rden_v, op=ALU.mult)

                # ---- per tile: diag(w_k) matmuls -> projected + gate ----
                for t in range(tn):
                    ti = s0 + t
                    bs = slice(t * P, (t + 1) * P)
                    g16 = g_tiles[t].bitcast(FP16)  # [P, TOPK, 2D]

                    diag4 = wpool.tile([P, TOPK, P], FP16, tag="diag4")
                    for k in range(TOPK):
                        nc.vector.tensor_scalar_mul(
                            diag4[:, k, :], ident_sb, w_g[:, t * 4 + k : t * 4 + k + 1]
                        )

                    # projected[b, :] = sum_k w_k[b] * Vp[idx_k[b], :]
                    ps_o = pp_o.tile([P, D], F32, tag="o")
                    for k in range(TOPK):
                        nc.tensor.matmul(
                            ps_o,
                            diag4[:, k, :],
                            g16[:, k, 0:D],
                            start=(k == 0),
                            stop=(k == TOPK - 1),
                        )
                    proj_sb = wpool.tile([P, D], FP16, tag="proj")
                    nc.scalar.copy(proj_sb, ps_o)

                    # Z = query @ Wg1.T + sum_k w_k * (Vg[idx_k] + bg)
                    # Z1 as 3 fp8 DR passes: q8.wg8 + qr.wg8/16 + q/16.wgr*16
                    ps_z = pp_z.tile([P, D], F32, tag="z")
                    first = True
                    for qa, wb in ((q8_g, wg8_sb), (qr_g, wg84_sb), (q4_g, wgr_sb)):
                        for cc in range(2):
                            nc.tensor.matmul(
                                ps_z,
                                qa[:, cc, :, bs],
                                wb[:, cc, :, :],
                                start=first,
                                stop=False,
                                perf_mode=DR,
                            )
                            first = False
                    for k in range(TOPK):
                        nc.tensor.matmul(
                            ps_z,
                            diag4[:, k, :],
                            g16[:, k, D : 2 * D],
                            start=False,
                            stop=(k == TOPK - 1),
                        )
                    gate_sb = wpool.tile([P, D], FP16, tag="gate")
                    nc.scalar.activation(gate_sb, ps_z, AF.Sigmoid)

                    out_sb = wpool.tile([P, D], FP16, tag="outb")
                    nc.vector.tensor_mul(out_sb, gate_sb, proj_sb)
                    nc.sync.dma_start(
                        out=out_d.ap()[ti * P : (ti + 1) * P, :], in_=out_sb
                    )

    nc.compile()
    return nc


def _host_prep(query, episode_keys, episode_values, Wq, Wv, Wg, bg):
    """Fold constants in fp64 and stage per-core device inputs."""
    kn = episode_keys.astype(np.float64)
    kn = kn / np.maximum(np.linalg.norm(kn, axis=1, keepdims=True), 1e-12)
    wc64 = Wq.astype(np.float64).T @ kn.T  # [D, E]
    wc_hi = round_fp32r(wc64.astype(np.float32))
    wc_lo = (wc64 - wc_hi.astype(np.float64)).astype(np.float32)

    q = np.ascontiguousarray(query, dtype=np.float32)
    q_hi = round_fp32r(q)
    q_lo = q - q_hi
    qT = np.ascontiguousarray(q.T)  # [D, B]
    qT_hi = np.ascontiguousarray(q_hi.T)
    qT_ls = np.ascontiguousarray((q_lo.T * FS)).astype(E4)
    qT_hs = np.ascontiguousarray((qT_hi * (1.0 / FS))).astype(E5)
    qT_8 = qT.astype(E4)
    qT_r = ((qT - qT_8.astype(np.float32)) * 16.0).astype(E5)
    qT_4 = (qT * (1.0 / 16.0)).astype(E5)

    v64 = episode_values.astype(np.float64)
    vp64 = v64 @ Wv.astype(np.float64).T                  # projected values
    vg64 = vp64 @ Wg.astype(np.float64)[:, D:].T          # gate Z2 values
    vg64 = vg64 + bg.astype(np.float64)[None, :]          # bg folded (sum w = 1)
    vpg16 = np.ascontiguousarray(
        np.concatenate(
            [vp64.astype(np.float16), vg64.astype(np.float16)], axis=1
        )
    )  # [E, 2D] fp16
    wg1t = np.ascontiguousarray(Wg.T[:D], dtype=np.float32)  # [D, D]
    wg8 = wg1t.astype(E5)
    consts = {
        "Wc_hi": np.ascontiguousarray(wc_hi),
        "Wc_hs": (wc_hi * (1.0 / FS)).astype(E5),
        "Wc_ls": (wc_lo * FS).astype(E5),
        "WqT8": np.ascontiguousarray(Wq.T).astype(E4),
        "Wg8": wg8,
        "Wg84": (wg1t * (1.0 / 16.0)).astype(E5),
        "Wgr": ((wg1t - wg8.astype(np.float32)) * 16.0).astype(E5),
        "ident": np.eye(P, dtype=np.float16),
        "rep16": np.tile(np.eye(16, dtype=np.float32), (1, P // 16)).reshape(16, P),
        "Vpg32": vpg16.view(np.int32),
    }
    return qT_hi, qT_ls, qT_hs, qT_8, qT_r, qT_4, consts


_PROGRAM_CACHE: dict = {}


def kernel(query, episode_keys, episode_values, Wq, Wv, Wg, bg, top_k):
    assert int(top_k) == TOPK
    query = np.asarray(query, dtype=np.float32)
    assert query.shape == (B_FULL, D), query.shape

    nt = B_FULL // N_CORES // P  # 16 tiles per core
    if nt not in _PROGRAM_CACHE:
        _PROGRAM_CACHE[nt] = build_program(nt)
    nc = _PROGRAM_CACHE[nt]

    qT_hi, qT_ls, qT_hs, qT_8, qT_r, qT_4, consts = _host_prep(
        query,
        np.asarray(episode_keys, dtype=np.float32),
        np.asarray(episode_values, dtype=np.float32),
        np.asarray(Wq, dtype=np.float32),
        np.asarray(Wv, dtype=np.float32),
        np.asarray(Wg, dtype=np.float32),
        np.asarray(bg, dtype=np.float32),
    )

    bc = B_FULL // N_CORES
    in_maps = []
    for c in range(N_CORES):
        cs = slice(c * bc, (c + 1) * bc)
        m = dict(consts)
        m["qT_hi"] = np.ascontiguousarray(qT_hi[:, cs])
        m["qT_ls"] = np.ascontiguousarray(qT_ls[:, cs])
        m["qT_hs"] = np.ascontiguousarray(qT_hs[:, cs])
        m["qT_8"] = np.ascontiguousarray(qT_8[:, cs])
        m["qT_r"] = np.ascontiguousarray(qT_r[:, cs])
        m["qT_4"] = np.ascontiguousarray(qT_4[:, cs])
        in_maps.append(m)

    res = run_bass_kernel_spmd(nc, in_maps, list(range(N_CORES)))
    global _LAST_RUN
    _LAST_RUN = res
    out = np.concatenate([res.results[c]["out"] for c in range(N_CORES)], axis=0)
    return out.astype(np.float32)


_LAST_RUN = None

